# revision 1
# baseline (speedup 1.0000x reference)
"""Trainium2 Bass kernel for ConditionalAttentionFusion-v2.

Math (per batch b, channel c, pixel y,x):
    CD   = concat(rgb_var, d_var)                       # [2,H,W], shared
    AB   = Wp[c,0]*rgb + Wp[c,1]*d
    CDc  = conv3x3(CD, W_unc[c])                        # 2-in 1-out per channel
    G    = Wt[c,0]*AB + Wt[c,1]*CDc
    out  = rgb*G + d*(1-G) = d + (rgb-d)*G

Strategy: pure data parallel over 8 cores (core = (batch, H-half), slab of 256
rows).  On each core the 3x3 conv (y-taps) + per-channel 1x1 terms are computed
on the TensorEngine as banded/diagonal-matrix matmuls accumulating into PSUM:

    G[r, x] = sum_{i,kx} Band[c,i,kx].T @ V_i[:, x+kx]    (6 band matmuls)
            + diag(a0[c]).T @ rgb + diag(a1[c]).T @ d     (2 diag matmuls)

where Band[p=r+ky, m=r] = Wt[c,1]*W_unc[c,i,ky,kx] folds the three ky taps of
the conv into one matmul (output rows 0..125 valid per 128-row V tile).  The
x-shifts (kx) are free-dim offsets into an x-padded V tile; the y-halo is
handled host-side by padding the var slab.  VectorE then does the 3-op tail:
diff = rgb-d; P = diff*G(PSUM); out = P + d.

A slab of 256 rows = two 126-row band tiles + a 4-row remainder.  The
remainder stacks all 19 channels into one matmul group (output partition
m = 4c+r), so it costs only 6 band + 2 diag matmuls total.

All band/diag matrices are precomputed host-side in numpy from the runtime
weight tensors and passed as extra kernel inputs.

Precision: the band (conv) matmuls use float32r (single-pass, bf16-array
speed) since the conv term is small; the diag matmuls on rgb/d use exact
float32 (2-pass) since those terms dominate the output magnitude.  Measured
absmax error ~1.7e-3 on an output scale of ~26 (6.5e-5 scale-relative).
"""
import sys

if "/opt/trn_rl_repo" not in sys.path:
    sys.path.insert(0, "/opt/trn_rl_repo")

import numpy as np

import concourse.bacc as bacc
import concourse.mybir as mybir
import concourse.tile as tile
from concourse.bass_utils import run_bass_kernel_spmd

F32 = mybir.dt.float32
F32R = mybir.dt.float32r
B, C, H, W = 4, 19, 512, 1024
R = 256              # slab rows per core
NCORES = 8
MAIN_Y0 = (0, 126)   # 126-row band tiles
REM_Y0 = 252         # 4-row remainder, stacked over channels


# ----------------------------------------------------------------- host math
def _build_mats(W_prob, W_unc, W_total):
    a0 = W_total[:, 0] * W_prob[:, 0]
    a1 = W_total[:, 0] * W_prob[:, 1]
    Wp = W_total[:, 1][:, None, None, None] * W_unc          # [C,2,3,3]

    bands = np.zeros((C, 128, 6, 128), np.float32)           # [c,p,s,m]
    r = np.arange(126)
    for i in range(2):
        for kx in range(3):
            s = i * 3 + kx
            for ky in range(3):
                bands[:, r + ky, s, r] = Wp[:, i, ky, kx][:, None]

    diags = np.zeros((C, 128, 2, 128), np.float32)           # [c,p,j,m]
    m = np.arange(128)
    diags[:, m, 0, m] = a0[:, None]
    diags[:, m, 1, m] = a1[:, None]

    remb = np.zeros((6, 6, 128), np.float32)                 # [p,s,m], m=4c+r
    rr = np.arange(4)
    for i in range(2):
        for kx in range(3):
            s = i * 3 + kx
            for ky in range(3):
                for c in range(C):
                    remb[rr + ky, s, 4 * c + rr] = Wp[c, i, ky, kx]

    remd = np.zeros((76, 2, 76), np.float32)                 # [p,j,m], p=m=4c+r
    p = np.arange(76)
    remd[p, 0, p] = np.repeat(a0, 4)
    remd[p, 1, p] = np.repeat(a1, 4)

    return (bands.reshape(C, 128, 768), diags.reshape(C, 128, 256),
            remb.reshape(6, 768), remd.reshape(76, 152))


# ------------------------------------------------------------- bass program
_CACHE = {}


def _build_program():
    nc = bacc.Bacc("TRN2", debug=False, num_devices=NCORES)
    f = F32R
    rgb_s = nc.dram_tensor("rgb_s", [C, R, W], F32, kind="ExternalInput").ap()
    d_s = nc.dram_tensor("d_s", [C, R, W], F32, kind="ExternalInput").ap()
    var_s = nc.dram_tensor("var_s", [2, R + 2, W + 2], f, kind="ExternalInput").ap()
    bands = nc.dram_tensor("bands", [C, 128, 768], f, kind="ExternalInput").ap()
    diags = nc.dram_tensor("diags", [C, 128, 256], F32, kind="ExternalInput").ap()
    remb = nc.dram_tensor("remb", [6, 768], f, kind="ExternalInput").ap()
    remd = nc.dram_tensor("remd", [76, 152], F32, kind="ExternalInput").ap()
    out_s = nc.dram_tensor("out_s", [C, R, W], F32, kind="ExternalOutput").ap()

    with tile.TileContext(nc) as tc:
        with (
            tc.tile_pool(name="wpool", bufs=1) as wpool,
            tc.tile_pool(name="vpool", bufs=4) as vpool,
            tc.tile_pool(name="io", bufs=3) as io,
            tc.tile_pool(name="tmp", bufs=2) as tmp,
            tc.tile_pool(name="psum", bufs=4, space="PSUM") as psum,
        ):
            band_sb, diag_sb = [], []
            for c in range(C):
                bt = wpool.tile([128, 768], f, tag=f"band{c}", name=f"band{c}")
                nc.sync.dma_start(out=bt[:], in_=bands[c])
                dt_ = wpool.tile([128, 256], F32, tag=f"diag{c}", name=f"diag{c}")
                nc.sync.dma_start(out=dt_[:], in_=diags[c])
                band_sb.append(bt)
                diag_sb.append(dt_)
            remb_sb = wpool.tile([6, 768], f, tag="remb", name="remb_sb")
            nc.sync.dma_start(out=remb_sb[:], in_=remb[:])
            remd_sb = wpool.tile([76, 152], F32, tag="remd", name="remd_sb")
            nc.sync.dma_start(out=remd_sb[:], in_=remd[:])

            # ---------------- main 126-row band tiles
            for y0 in MAIN_Y0:
                vt = []
                for i in range(2):
                    v = vpool.tile([128, W + 2], f, tag="v", name=f"v{i}_{y0}")
                    nc.sync.dma_start(out=v[:], in_=var_s[i, y0:y0 + 128, :])
                    vt.append(v)
                for c in range(C):
                    rt = io.tile([126, W], F32, tag="r", name=f"r{y0}_{c}")
                    nc.sync.dma_start(out=rt[:], in_=rgb_s[c, y0:y0 + 126, :])
                    dt = io.tile([126, W], F32, tag="d", name=f"d{y0}_{c}")
                    nc.sync.dma_start(out=dt[:], in_=d_s[c, y0:y0 + 126, :])

                    ps = psum.tile([128, W], F32, tag="ps", name=f"ps{y0}_{c}")
                    for xb in (0, 512):
                        for s in range(6):
                            i, kx = divmod(s, 3)
                            nc.tensor.matmul(
                                ps[:, xb:xb + 512],
                                band_sb[c][:, s * 128:(s + 1) * 128],
                                vt[i][:, xb + kx:xb + kx + 512],
                                start=(s == 0), stop=False)
                        nc.tensor.matmul(
                            ps[:126, xb:xb + 512],
                            diag_sb[c][:126, 0:126],
                            rt[:, xb:xb + 512], start=False, stop=False)
                        nc.tensor.matmul(
                            ps[:126, xb:xb + 512],
                            diag_sb[c][:126, 128:254],
                            dt[:, xb:xb + 512], start=False, stop=True)

                    diff = tmp.tile([126, W], F32, tag="diff", name=f"diff{y0}_{c}")
                    nc.vector.tensor_sub(out=diff[:], in0=rt[:], in1=dt[:])
                    prod = tmp.tile([126, W], F32, tag="prod", name=f"prod{y0}_{c}")
                    nc.vector.tensor_mul(out=prod[:], in0=diff[:], in1=ps[:126, :])
                    ot = io.tile([126, W], F32, tag="o", name=f"o{y0}_{c}")
                    nc.vector.tensor_add(out=ot[:], in0=prod[:], in1=dt[:])
                    nc.sync.dma_start(out=out_s[c, y0:y0 + 126, :], in_=ot[:])

            # ---------------- 4-row remainder, all channels stacked (m = 4c+r)
            vr = []
            for i in range(2):
                v = vpool.tile([6, W + 2], f, tag=f"vrem{i}", name=f"vrem{i}", bufs=1)
                nc.sync.dma_start(out=v[:], in_=var_s[i, REM_Y0:REM_Y0 + 6, :])
                vr.append(v)
            rr = io.tile([76, W], F32, tag="rrem", name="rrem", bufs=1)
            dr = io.tile([76, W], F32, tag="drem", name="drem", bufs=1)
            for c in range(C):
                nc.sync.dma_start(out=rr[4 * c:4 * c + 4, :],
                                  in_=rgb_s[c, REM_Y0:REM_Y0 + 4, :])
                nc.sync.dma_start(out=dr[4 * c:4 * c + 4, :],
                                  in_=d_s[c, REM_Y0:REM_Y0 + 4, :])
            ps = psum.tile([128, W], F32, tag="ps", name="ps_rem")
            for xb in (0, 512):
                for s in range(6):
                    i, kx = divmod(s, 3)
                    nc.tensor.matmul(
                        ps[:, xb:xb + 512],
                        remb_sb[:, s * 128:(s + 1) * 128],
                        vr[i][:, xb + kx:xb + kx + 512],
                        start=(s == 0), stop=False)
                nc.tensor.matmul(ps[:76, xb:xb + 512], remd_sb[:, 0:76],
                                 rr[:, xb:xb + 512], start=False, stop=False)
                nc.tensor.matmul(ps[:76, xb:xb + 512], remd_sb[:, 76:152],
                                 dr[:, xb:xb + 512], start=False, stop=True)
            diff = tmp.tile([76, W], F32, tag="diffrem", name="diff_rem", bufs=1)
            nc.vector.tensor_sub(out=diff[:], in0=rr[:], in1=dr[:])
            prod = tmp.tile([76, W], F32, tag="prodrem", name="prod_rem", bufs=1)
            nc.vector.tensor_mul(out=prod[:], in0=diff[:], in1=ps[:76, :])
            ot = io.tile([76, W], F32, tag="orem", name="o_rem", bufs=1)
            nc.vector.tensor_add(out=ot[:], in0=prod[:], in1=dr[:])
            for c in range(C):
                nc.sync.dma_start(out=out_s[c, REM_Y0:REM_Y0 + 4, :],
                                  in_=ot[4 * c:4 * c + 4, :])

    nc.compile()
    return nc


def _shard_inputs(rgb, d, rgb_var, d_var, W_prob, W_unc, W_total):
    bands, diags, remb, remd = _build_mats(
        np.asarray(W_prob, np.float32),
        np.asarray(W_unc, np.float32),
        np.asarray(W_total, np.float32))
    in_maps = []
    for core in range(NCORES):
        b, half = divmod(core, 2)
        h0 = half * R
        var = np.zeros((2, R + 2, W + 2), np.float32)
        lo, hi = max(h0 - 1, 0), min(h0 + R + 1, H)
        var[0, lo - h0 + 1:hi - h0 + 1, 1:W + 1] = rgb_var[b, 0, lo:hi, :]
        var[1, lo - h0 + 1:hi - h0 + 1, 1:W + 1] = d_var[b, 0, lo:hi, :]
        in_maps.append({
            "rgb_s": np.ascontiguousarray(rgb[b, :, h0:h0 + R, :], np.float32),
            "d_s": np.ascontiguousarray(d[b, :, h0:h0 + R, :], np.float32),
            "var_s": var,
            "bands": bands, "diags": diags, "remb": remb, "remd": remd,
        })
    return in_maps


def run(trace=False, **inputs):
    if "nc" not in _CACHE:
        _CACHE["nc"] = _build_program()
    nc = _CACHE["nc"]
    in_maps = _shard_inputs(**inputs)
    res = run_bass_kernel_spmd(nc, in_maps, list(range(NCORES)), trace=trace)
    out = np.empty((B, C, H, W), np.float32)
    for core in range(NCORES):
        b, half = divmod(core, 2)
        out[b, :, half * R:(half + 1) * R, :] = res.results[core]["out_s"]
    return out, res


def kernel(**inputs):
    out, _ = run(trace=False, **inputs)
    return out



# revision 2
# speedup vs baseline: 1.0277x; 1.0277x over previous
"""Trainium2 Bass kernel for ConditionalAttentionFusion-v2.

Math (per batch b, channel c, pixel y,x):
    CD   = concat(rgb_var, d_var)                       # [2,H,W], shared
    AB   = Wp[c,0]*rgb + Wp[c,1]*d
    CDc  = conv3x3(CD, W_unc[c])                        # 2-in 1-out per channel
    G    = Wt[c,0]*AB + Wt[c,1]*CDc
    out  = rgb*G + d*(1-G) = d + (rgb-d)*G

Strategy: pure data parallel over 8 cores (core = (batch, H-half), slab of 256
rows).  The kernel is HBM-bandwidth bound, so rgb/d/out move as bf16 (halves
traffic; rel-err ~8e-3 vs the 2e-2 gate).  Per core:

  - conv term on TensorE: banded-matrix matmuls (3 ky taps folded into one
    bf16 [128,128] stationary per (i,kx); 6 matmuls per 512-col block)
    accumulating Wt1*CDc into PSUM.
  - diag terms on VectorE: per-channel fused scalar_tensor_tensor
    g = a0*rgb + (a1*d + g_conv), with a0=Wt0*Wp0, a1=Wt0*Wp1 as
    per-partition scalars.
  - ScalarE drains PSUM -> bf16 SBUF; GPSIMD does the chunk-wide
    diff = rgb-d and prod = diff*G; VectorE the final out = prod + d.
  - loads are issued on the sync HWDGE queue, stores on the scalar HWDGE
    queue; rgb/d/out travel in ~1MB DMAs (4-channel chunks, [R,C,W] layout
    host-side so a chunk is one multi-descriptor DMA).

Slab = two 126-row band tiles + a 4-row remainder where all channels are
stacked in the partition dim (p = r*CP + c), so the remainder costs one
matmul group + one vector-op group total.
"""
import sys

if "/opt/trn_rl_repo" not in sys.path:
    sys.path.insert(0, "/opt/trn_rl_repo")

import numpy as np

import concourse.bacc as bacc
import concourse.mybir as mybir
import concourse.tile as tile
from concourse.bass_utils import run_bass_kernel_spmd

F32 = mybir.dt.float32
BF16 = mybir.dt.bfloat16
NP_BF16 = mybir.dt.np(mybir.dt.bfloat16)
ALU = mybir.AluOpType
ACT_COPY = mybir.ActivationFunctionType.Copy

B, C, H, W = 4, 19, 512, 1024
CP = 20              # channels padded to a multiple of the chunk size
CG = 4               # channels per IO chunk
R = 256              # slab rows per core
NCORES = 8
MAIN_Y0 = (0, 126)   # 126-row band tiles
REM_Y0 = 252         # 4-row remainder, stacked over channels


# ----------------------------------------------------------------- host math
def _build_mats(W_prob, W_unc, W_total):
    a0 = np.zeros(CP, np.float32)
    a1 = np.zeros(CP, np.float32)
    a0[:C] = W_total[:, 0] * W_prob[:, 0]
    a1[:C] = W_total[:, 0] * W_prob[:, 1]
    Wp = np.zeros((CP, 2, 3, 3), np.float32)
    Wp[:C] = W_total[:, 1][:, None, None, None] * W_unc      # [C,2,3,3]

    bands = np.zeros((CP, 128, 6, 128), np.float32)          # [c,p,s,m]
    r = np.arange(126)
    for i in range(2):
        for kx in range(3):
            s = i * 3 + kx
            for ky in range(3):
                bands[:, r + ky, s, r] = Wp[:, i, ky, kx][:, None]

    remb = np.zeros((6, 6, 128), np.float32)                 # [p,s,m], m=r*CP+c
    rr = np.arange(4)
    for i in range(2):
        for kx in range(3):
            s = i * 3 + kx
            for ky in range(3):
                for c in range(CP):
                    remb[rr + ky, s, rr * CP + c] = Wp[c, i, ky, kx]

    # per-partition scalar tables: a0m[p, c] == a0[c]  (any p)
    a0m = np.tile(a0[None, :], (128, 1)).astype(np.float32)  # [128, CP]
    a1m = np.tile(a1[None, :], (128, 1)).astype(np.float32)
    a0r = np.tile(a0, 4)[:, None].astype(np.float32)         # [4*CP, 1], p=r*CP+c
    a1r = np.tile(a1, 4)[:, None].astype(np.float32)

    return (bands.reshape(CP, 128, 768).astype(NP_BF16),
            remb.reshape(6, 768).astype(NP_BF16), a0m, a1m, a0r, a1r)


# ------------------------------------------------------------- bass program
_CACHE = {}


def _build_program():
    nc = bacc.Bacc("TRN2", debug=False, num_devices=NCORES)
    rgb_s = nc.dram_tensor("rgb_s", [R, CP, W], BF16, kind="ExternalInput").ap()
    d_s = nc.dram_tensor("d_s", [R, CP, W], BF16, kind="ExternalInput").ap()
    var_s = nc.dram_tensor("var_s", [2, R + 2, W + 2], BF16, kind="ExternalInput").ap()
    bands = nc.dram_tensor("bands", [CP, 128, 768], BF16, kind="ExternalInput").ap()
    remb = nc.dram_tensor("remb", [6, 768], BF16, kind="ExternalInput").ap()
    a0m_d = nc.dram_tensor("a0m", [128, CP], F32, kind="ExternalInput").ap()
    a1m_d = nc.dram_tensor("a1m", [128, CP], F32, kind="ExternalInput").ap()
    a0r_d = nc.dram_tensor("a0r", [4 * CP, 1], F32, kind="ExternalInput").ap()
    a1r_d = nc.dram_tensor("a1r", [4 * CP, 1], F32, kind="ExternalInput").ap()
    out_s = nc.dram_tensor("out_s", [R, CP, W], BF16, kind="ExternalOutput").ap()

    NCH = CP // CG  # chunks per row-block

    with tile.TileContext(nc) as tc:
        with (
            tc.tile_pool(name="wpool", bufs=1) as wpool,
            tc.tile_pool(name="vpool", bufs=2) as vpool,
            tc.tile_pool(name="io", bufs=2) as io,
            tc.tile_pool(name="tmp", bufs=2) as tmp,
            tc.tile_pool(name="psum", bufs=3, space="PSUM") as psum,
        ):
            band_sb = []
            for c in range(CP):
                bt = wpool.tile([128, 768], BF16, tag=f"band{c}", name=f"band{c}")
                nc.sync.dma_start(out=bt[:], in_=bands[c])
                band_sb.append(bt)
            remb_sb = wpool.tile([6, 768], BF16, tag="remb", name="remb_sb")
            nc.sync.dma_start(out=remb_sb[:], in_=remb[:])
            a0m = wpool.tile([128, CP], F32, tag="a0m", name="a0m")
            nc.sync.dma_start(out=a0m[:], in_=a0m_d[:])
            a1m = wpool.tile([128, CP], F32, tag="a1m", name="a1m")
            nc.sync.dma_start(out=a1m[:], in_=a1m_d[:])
            a0r = wpool.tile([4 * CP, 1], F32, tag="a0r", name="a0r")
            nc.sync.dma_start(out=a0r[:], in_=a0r_d[:])
            a1r = wpool.tile([4 * CP, 1], F32, tag="a1r", name="a1r")
            nc.sync.dma_start(out=a1r[:], in_=a1r_d[:])

            # ---------------- main 126-row band tiles
            for y0 in MAIN_Y0:
                vt = []
                for i in range(2):
                    v = vpool.tile([128, W + 2], BF16, tag=f"v{i}", name=f"v{i}_{y0}")
                    nc.sync.dma_start(out=v[:], in_=var_s[i, y0:y0 + 128, :])
                    vt.append(v)
                for g0 in range(NCH):
                    c0 = g0 * CG
                    rt = io.tile([126, CG, W], BF16, tag="r", name=f"r{y0}_{g0}")
                    nc.sync.dma_start(out=rt[:], in_=rgb_s[y0:y0 + 126, c0:c0 + CG, :])
                    dt = io.tile([126, CG, W], BF16, tag="d", name=f"d{y0}_{g0}")
                    nc.sync.dma_start(out=dt[:], in_=d_s[y0:y0 + 126, c0:c0 + CG, :])

                    gt = tmp.tile([126, CG, W], BF16, tag="g", name=f"g{y0}_{g0}")
                    g2 = tmp.tile([126, CG, W], BF16, tag="g2", name=f"g2{y0}_{g0}")
                    for ci in range(CG):
                        c = c0 + ci
                        ps = psum.tile([128, W], F32, tag="ps", name=f"ps{y0}_{c}")
                        for xb in (0, 512):
                            for s in range(6):
                                i, kx = divmod(s, 3)
                                nc.tensor.matmul(
                                    ps[:, xb:xb + 512],
                                    band_sb[c][:, s * 128:(s + 1) * 128],
                                    vt[i][:, xb + kx:xb + kx + 512],
                                    start=(s == 0), stop=(s == 5))
                        # drain conv term to bf16
                        nc.scalar.activation(out=gt[:, ci, :], in_=ps[:126, :],
                                             func=ACT_COPY)
                        # g2 = a0*rgb + g ; g = a1*d + g2   (per-channel scalars)
                        nc.vector.scalar_tensor_tensor(
                            out=g2[:, ci, :], in0=rt[:, ci, :],
                            scalar=a0m[:126, c:c + 1], in1=gt[:, ci, :],
                            op0=ALU.mult, op1=ALU.add)
                        nc.vector.scalar_tensor_tensor(
                            out=gt[:, ci, :], in0=dt[:, ci, :],
                            scalar=a1m[:126, c:c + 1], in1=g2[:, ci, :],
                            op0=ALU.mult, op1=ALU.add)
                    # chunk-wide tail: out = d + (rgb-d)*G
                    df = tmp.tile([126, CG, W], BF16, tag="df", name=f"df{y0}_{g0}")
                    nc.gpsimd.tensor_tensor(out=df[:], in0=rt[:], in1=dt[:],
                                            op=ALU.subtract)
                    pr = tmp.tile([126, CG, W], BF16, tag="pr", name=f"pr{y0}_{g0}")
                    nc.gpsimd.tensor_tensor(out=pr[:], in0=df[:], in1=gt[:],
                                            op=ALU.mult)
                    ot = io.tile([126, CG, W], BF16, tag="o", name=f"o{y0}_{g0}")
                    nc.vector.tensor_tensor(out=ot[:], in0=pr[:], in1=dt[:],
                                            op=ALU.add)
                    nc.scalar.dma_start(out=out_s[y0:y0 + 126, c0:c0 + CG, :],
                                        in_=ot[:])

            # ---------------- 4-row remainder, all channels stacked (m = r*CP+c)
            P = 4 * CP
            vr = []
            for i in range(2):
                v = vpool.tile([6, W + 2], BF16, tag=f"vr{i}", name=f"vr{i}", bufs=1)
                nc.sync.dma_start(out=v[:], in_=var_s[i, REM_Y0:REM_Y0 + 6, :])
                vr.append(v)
            rr = io.tile([P, W], BF16, tag="rrem", name="rrem", bufs=1)
            nc.sync.dma_start(out=rr[:], in_=rgb_s[REM_Y0:REM_Y0 + 4, :, :])
            dr = io.tile([P, W], BF16, tag="drem", name="drem", bufs=1)
            nc.sync.dma_start(out=dr[:], in_=d_s[REM_Y0:REM_Y0 + 4, :, :])

            ps = psum.tile([128, W], F32, tag="psrem", name="ps_rem", bufs=1)
            for xb in (0, 512):
                for s in range(6):
                    i, kx = divmod(s, 3)
                    nc.tensor.matmul(
                        ps[:, xb:xb + 512],
                        remb_sb[:, s * 128:s * 128 + 128],
                        vr[i][:, xb + kx:xb + kx + 512],
                        start=(s == 0), stop=(s == 5))
            gt = tmp.tile([P, W], BF16, tag="grem", name="g_rem", bufs=1)
            nc.scalar.activation(out=gt[:], in_=ps[:P, :], func=ACT_COPY)
            g2 = tmp.tile([P, W], BF16, tag="g2rem", name="g2_rem", bufs=1)
            nc.vector.scalar_tensor_tensor(out=g2[:], in0=rr[:], scalar=a0r[:],
                                           in1=gt[:], op0=ALU.mult, op1=ALU.add)
            nc.vector.scalar_tensor_tensor(out=gt[:], in0=dr[:], scalar=a1r[:],
                                           in1=g2[:], op0=ALU.mult, op1=ALU.add)
            df = tmp.tile([P, W], BF16, tag="dfrem", name="df_rem", bufs=1)
            nc.gpsimd.tensor_tensor(out=df[:], in0=rr[:], in1=dr[:],
                                    op=ALU.subtract)
            pr = tmp.tile([P, W], BF16, tag="prrem", name="pr_rem", bufs=1)
            nc.gpsimd.tensor_tensor(out=pr[:], in0=df[:], in1=gt[:], op=ALU.mult)
            ot = io.tile([P, W], BF16, tag="orem", name="o_rem", bufs=1)
            nc.vector.tensor_tensor(out=ot[:], in0=pr[:], in1=dr[:], op=ALU.add)
            nc.scalar.dma_start(out=out_s[REM_Y0:REM_Y0 + 4, :, :], in_=ot[:])

    nc.compile()
    return nc


def _shard_inputs(rgb, d, rgb_var, d_var, W_prob, W_unc, W_total):
    bands, remb, a0m, a1m, a0r, a1r = _build_mats(
        np.asarray(W_prob, np.float32),
        np.asarray(W_unc, np.float32),
        np.asarray(W_total, np.float32))
    # host layout: [R, CP, W] bf16, channel-padded
    rgb_b = np.asarray(rgb, np.float32).astype(NP_BF16)
    d_b = np.asarray(d, np.float32).astype(NP_BF16)
    in_maps = []
    for core in range(NCORES):
        b, half = divmod(core, 2)
        h0 = half * R
        var = np.zeros((2, R + 2, W + 2), np.float32)
        lo, hi = max(h0 - 1, 0), min(h0 + R + 1, H)
        var[0, lo - h0 + 1:hi - h0 + 1, 1:W + 1] = rgb_var[b, 0, lo:hi, :]
        var[1, lo - h0 + 1:hi - h0 + 1, 1:W + 1] = d_var[b, 0, lo:hi, :]
        rgb_c = np.zeros((R, CP, W), NP_BF16)
        rgb_c[:, :C, :] = rgb_b[b, :, h0:h0 + R, :].transpose(1, 0, 2)
        d_c = np.zeros((R, CP, W), NP_BF16)
        d_c[:, :C, :] = d_b[b, :, h0:h0 + R, :].transpose(1, 0, 2)
        in_maps.append({
            "rgb_s": rgb_c, "d_s": d_c, "var_s": var.astype(NP_BF16),
            "bands": bands, "remb": remb,
            "a0m": a0m, "a1m": a1m, "a0r": a0r, "a1r": a1r,
        })
    return in_maps


def run(trace=False, **inputs):
    if "nc" not in _CACHE:
        _CACHE["nc"] = _build_program()
    nc = _CACHE["nc"]
    in_maps = _shard_inputs(**inputs)
    res = run_bass_kernel_spmd(nc, in_maps, list(range(NCORES)), trace=trace)
    out = np.empty((B, C, H, W), np.float32)
    for core in range(NCORES):
        b, half = divmod(core, 2)
        o = np.asarray(res.results[core]["out_s"])[:, :C, :]  # [R, C, W] bf16
        out[b, :, half * R:(half + 1) * R, :] = o.transpose(1, 0, 2).astype(np.float32)
    return out, res


def kernel(**inputs):
    out, _ = run(trace=False, **inputs)
    return out


# revision 8
# speedup vs baseline: 1.1771x; 1.1453x over previous
"""Trainium2 Bass kernel for ConditionalAttentionFusion-v2.

Math (per batch b, channel c, pixel y,x):
    CD   = concat(rgb_var, d_var)                       # [2,H,W], shared
    AB   = Wp[c,0]*rgb + Wp[c,1]*d
    CDc  = conv3x3(CD, W_unc[c])                        # 2-in 1-out per channel
    G    = Wt[c,0]*AB + Wt[c,1]*CDc
    out  = rgb*G + d*(1-G) = d + (rgb-d)*G

HBM-bound problem -> rgb/d/out move as bf16 (rel-err ~8e-3 vs the 2e-2 gate).

Layout: pure data parallel over 8 cores (core = (batch, H-half), slab of 256
rows).  Host ships rgb/d as [R, C, W] (y-major) so a "block" of 6 consecutive
image rows x all 19 channels is 114 *consecutive* rows of the flat [R*C, W]
matrix = one [114, 1024] SBUF tile with partition p = 6-local-row*19 + c.

Per 6-row block, G is accumulated entirely in PSUM by TensorE:
  - conv term: one matmul against a single reused [108,114] stationary that
    encodes all 18 (i,ky,kx) taps; the moving tile is an im2col matrix
    S2[g*18+o, x] = var[i, 6b+g+ky, x+kx], built for a whole 126-row half by
    18 big strided HBM DMAs (3.4x re-read of the tiny var tensor).
  - diag terms: bf16 diagonal-matrix matmuls diag(a0), diag(a1) against the
    rgb/d tiles themselves (a0=Wt0*Wp0, a1=Wt0*Wp1 per channel).
ScalarE drains PSUM->bf16; VectorE does 3 chunk-wide bf16 tensor_tensor ops
(2x perf mode) for out = d + (rgb-d)*G.  GPSIMD does no compute (it would
contend for the DVE SBUF port) - it only issues output-store DMAs so loads
(sync queue), d-loads/drains (scalar queue) and stores (gpsimd queue) spread
over three DMA queues.  IO DMAs are ~700KB (3 blocks per chunk).

The 4-row tail (rows 252..255) uses the same scheme with 4x19=76-partition
tiles and its own [72,76] conv stationary.
"""
import sys

if "/opt/trn_rl_repo" not in sys.path:
    sys.path.insert(0, "/opt/trn_rl_repo")

import numpy as np

import concourse.bacc as bacc
import concourse.mybir as mybir
import concourse.tile as tile
from concourse.bass_utils import run_bass_kernel_spmd

F32 = mybir.dt.float32
BF16 = mybir.dt.bfloat16
NP_BF16 = mybir.dt.np(mybir.dt.bfloat16)
ALU = mybir.AluOpType
ACT_COPY = mybir.ActivationFunctionType.Copy

B, C, H, W = 4, 19, 512, 1024
R = 256              # slab rows per core
NCORES = 8
GR = 6               # image rows per block
P = GR * C           # partitions per block = 114
NB = 42              # full blocks per slab (252 rows)
NBH = 21             # blocks per 126-row half
CB = 3               # blocks per IO chunk
REM_Y = 252          # remainder rows 252..255
PR_ = 4 * C          # remainder partitions = 76


# ----------------------------------------------------------------- host math
def _build_mats(W_prob, W_unc, W_total):
    a0 = (W_total[:, 0] * W_prob[:, 0]).astype(np.float32)
    a1 = (W_total[:, 0] * W_prob[:, 1]).astype(np.float32)
    Wp = (W_total[:, 1][:, None, None, None] * W_unc).astype(np.float32)

    # conv stationary [108, 114]: row g*18+(i*9+ky*3+kx) -> out g*19+c
    st = np.zeros((108, P), np.float32)
    for g in range(GR):
        for i in range(2):
            for ky in range(3):
                for kx in range(3):
                    o = i * 9 + ky * 3 + kx
                    st[g * 18 + o, g * 19:(g + 1) * 19] = Wp[:, i, ky, kx]
    # remainder conv stationary [72, 76]: row r*18+o -> out r*19+c
    str_ = np.zeros((72, PR_), np.float32)
    for r in range(4):
        for i in range(2):
            for ky in range(3):
                for kx in range(3):
                    o = i * 9 + ky * 3 + kx
                    str_[r * 18 + o, r * 19:(r + 1) * 19] = Wp[:, i, ky, kx]

    d0 = np.diag(np.tile(a0, GR)).astype(np.float32)         # [114,114]
    d1 = np.diag(np.tile(a1, GR)).astype(np.float32)
    d0r = np.diag(np.tile(a0, 4)).astype(np.float32)         # [76,76]
    d1r = np.diag(np.tile(a1, 4)).astype(np.float32)
    return (st.astype(NP_BF16), str_.astype(NP_BF16),
            d0.astype(NP_BF16), d1.astype(NP_BF16),
            d0r.astype(NP_BF16), d1r.astype(NP_BF16))


# ------------------------------------------------------------- bass program
_CACHE = {}


def _build_program():
    nc = bacc.Bacc("TRN2", debug=False, num_devices=NCORES)
    # flat [R*C, W] y-major view of the slab
    rgb_s = nc.dram_tensor("rgb_s", [R * C, W], BF16, kind="ExternalInput").ap()
    d_s = nc.dram_tensor("d_s", [R * C, W], BF16, kind="ExternalInput").ap()
    var_s = nc.dram_tensor("var_s", [2, R + 2, W + 2], BF16, kind="ExternalInput").ap()
    st_d = nc.dram_tensor("conv_st", [108, P], BF16, kind="ExternalInput").ap()
    str_d = nc.dram_tensor("conv_str", [72, PR_], BF16, kind="ExternalInput").ap()
    d0_d = nc.dram_tensor("diag0", [P, P], BF16, kind="ExternalInput").ap()
    d1_d = nc.dram_tensor("diag1", [P, P], BF16, kind="ExternalInput").ap()
    d0r_d = nc.dram_tensor("diag0r", [PR_, PR_], BF16, kind="ExternalInput").ap()
    d1r_d = nc.dram_tensor("diag1r", [PR_, PR_], BF16, kind="ExternalInput").ap()
    out_s = nc.dram_tensor("out_s", [R * C, W], BF16, kind="ExternalOutput").ap()

    with tile.TileContext(nc) as tc:
        with (
            tc.tile_pool(name="wpool", bufs=1) as wpool,
            tc.tile_pool(name="spool", bufs=2) as spool,
            tc.tile_pool(name="io", bufs=2) as io,
            tc.tile_pool(name="tmp", bufs=2) as tmp,
            tc.tile_pool(name="psum", bufs=3, space="PSUM") as psum,
        ):
            st_sb = wpool.tile([108, P], BF16, tag="st", name="st_sb")
            nc.sync.dma_start(out=st_sb[:], in_=st_d[:])
            str_sb = wpool.tile([72, PR_], BF16, tag="str", name="str_sb")
            nc.sync.dma_start(out=str_sb[:], in_=str_d[:])
            d0_sb = wpool.tile([P, P], BF16, tag="d0", name="d0_sb")
            nc.sync.dma_start(out=d0_sb[:], in_=d0_d[:])
            d1_sb = wpool.tile([P, P], BF16, tag="d1", name="d1_sb")
            nc.sync.dma_start(out=d1_sb[:], in_=d1_d[:])
            d0r_sb = wpool.tile([PR_, PR_], BF16, tag="d0r", name="d0r_sb")
            nc.sync.dma_start(out=d0r_sb[:], in_=d0r_d[:])
            d1r_sb = wpool.tile([PR_, PR_], BF16, tag="d1r", name="d1r_sb")
            nc.sync.dma_start(out=d1r_sb[:], in_=d1r_d[:])

            for half in range(2):
                y0 = 126 * half
                # im2col for the half: S2[g*18+(i*9+ky*3+kx), b, x]
                #   = var_s[i, y0 + 6b + g + ky, x + kx]
                s2 = spool.tile([108, NBH, W], BF16, tag="s2", name=f"s2_{half}")
                for i in range(2):
                    for ky in range(3):
                        for kx in range(3):
                            o = i * 9 + ky * 3 + kx
                            src = var_s[i, y0 + ky:y0 + ky + 126,
                                        kx:kx + W].rearrange(
                                            "(b g) x -> g b x", g=GR)
                            nc.sync.dma_start(out=s2[o:90 + o + 1:18, :, :],
                                              in_=src)
                for j in range(NBH // CB):
                    b0 = half * NBH + j * CB          # global block id
                    r0 = b0 * P                       # flat row
                    rt = io.tile([P, CB, W], BF16, tag="r", name=f"r{b0}")
                    nc.sync.dma_start(
                        out=rt[:],
                        in_=rgb_s[r0:r0 + CB * P, :].rearrange(
                            "(blk p) x -> p blk x", p=P))
                    dt = io.tile([P, CB, W], BF16, tag="d", name=f"d{b0}")
                    nc.scalar.dma_start(
                        out=dt[:],
                        in_=d_s[r0:r0 + CB * P, :].rearrange(
                            "(blk p) x -> p blk x", p=P))

                    pss = []
                    for k in range(CB):
                        ps = psum.tile([P, W], F32, tag="ps", name=f"ps{b0 + k}")
                        pss.append(ps)
                    bl0 = j * CB                      # block index within half
                    for xb in (0, 512):
                        for k in range(CB):
                            nc.tensor.matmul(
                                pss[k][:, xb:xb + 512], st_sb[:],
                                s2[:, bl0 + k, xb:xb + 512],
                                start=True, stop=False)
                    for xb in (0, 512):
                        for k in range(CB):
                            nc.tensor.matmul(
                                pss[k][:, xb:xb + 512], d0_sb[:],
                                rt[:, k, xb:xb + 512], start=False, stop=False)
                    for xb in (0, 512):
                        for k in range(CB):
                            nc.tensor.matmul(
                                pss[k][:, xb:xb + 512], d1_sb[:],
                                dt[:, k, xb:xb + 512], start=False,
                                stop=True)
                    gt = tmp.tile([P, CB, W], BF16, tag="g", name=f"g{b0}")
                    for k in range(CB):
                        nc.scalar.activation(out=gt[:, k, :], in_=pss[k][:],
                                             func=ACT_COPY)
                    df = tmp.tile([P, CB, W], BF16, tag="df", name=f"df{b0}")
                    nc.vector.tensor_tensor(out=df[:], in0=rt[:], in1=dt[:],
                                            op=ALU.subtract)
                    pr = tmp.tile([P, CB, W], BF16, tag="pr", name=f"pr{b0}")
                    nc.vector.tensor_tensor(out=pr[:], in0=df[:], in1=gt[:],
                                            op=ALU.mult)
                    ot = io.tile([P, CB, W], BF16, tag="o", name=f"o{b0}")
                    nc.vector.tensor_tensor(out=ot[:], in0=pr[:], in1=dt[:],
                                            op=ALU.add)
                    nc.gpsimd.dma_start(
                        out=out_s[r0:r0 + CB * P, :].rearrange(
                            "(blk p) x -> p blk x", p=P),
                        in_=ot[:])

            # ---------------- 4-row remainder (rows 252..255), p = r*19+c
            s2r = spool.tile([72, W], BF16, tag="s2r", name="s2r", bufs=1)
            for i in range(2):
                for ky in range(3):
                    for kx in range(3):
                        o = i * 9 + ky * 3 + kx
                        nc.sync.dma_start(
                            out=s2r[o:54 + o + 1:18, :],
                            in_=var_s[i, REM_Y + ky:REM_Y + ky + 4,
                                      kx:kx + W])
            r0 = REM_Y * C
            rr = io.tile([PR_, W], BF16, tag="rr", name="rrem", bufs=1)
            nc.sync.dma_start(out=rr[:], in_=rgb_s[r0:r0 + PR_, :])
            dr = io.tile([PR_, W], BF16, tag="dr", name="drem", bufs=1)
            nc.scalar.dma_start(out=dr[:], in_=d_s[r0:r0 + PR_, :])
            ps = psum.tile([PR_, W], F32, tag="psr", name="ps_rem", bufs=1)
            for xb in (0, 512):
                nc.tensor.matmul(ps[:, xb:xb + 512], str_sb[:],
                                 s2r[:, xb:xb + 512],
                                 start=True, stop=False)
            for xb in (0, 512):
                nc.tensor.matmul(ps[:, xb:xb + 512], d0r_sb[:],
                                 rr[:, xb:xb + 512], start=False, stop=False)
            for xb in (0, 512):
                nc.tensor.matmul(ps[:, xb:xb + 512], d1r_sb[:],
                                 dr[:, xb:xb + 512], start=False,
                                 stop=True)
            gt = tmp.tile([PR_, W], BF16, tag="gr", name="g_rem", bufs=1)
            nc.scalar.activation(out=gt[:], in_=ps[:], func=ACT_COPY)
            df = tmp.tile([PR_, W], BF16, tag="dfr", name="df_rem", bufs=1)
            nc.vector.tensor_tensor(out=df[:], in0=rr[:], in1=dr[:],
                                    op=ALU.subtract)
            pr = tmp.tile([PR_, W], BF16, tag="prr", name="pr_rem", bufs=1)
            nc.vector.tensor_tensor(out=pr[:], in0=df[:], in1=gt[:],
                                    op=ALU.mult)
            ot = io.tile([PR_, W], BF16, tag="or", name="o_rem", bufs=1)
            nc.vector.tensor_tensor(out=ot[:], in0=pr[:], in1=dr[:],
                                    op=ALU.add)
            nc.gpsimd.dma_start(out=out_s[r0:r0 + PR_, :], in_=ot[:])

    nc.compile()
    return nc


def _shard_inputs(rgb, d, rgb_var, d_var, W_prob, W_unc, W_total):
    st, str_, d0, d1, d0r, d1r = _build_mats(
        np.asarray(W_prob, np.float32),
        np.asarray(W_unc, np.float32),
        np.asarray(W_total, np.float32))
    rgb_b = np.asarray(rgb, np.float32).astype(NP_BF16)
    d_b = np.asarray(d, np.float32).astype(NP_BF16)
    in_maps = []
    for core in range(NCORES):
        b, half = divmod(core, 2)
        h0 = half * R
        var = np.zeros((2, R + 2, W + 2), np.float32)
        lo, hi = max(h0 - 1, 0), min(h0 + R + 1, H)
        var[0, lo - h0 + 1:hi - h0 + 1, 1:W + 1] = rgb_var[b, 0, lo:hi, :]
        var[1, lo - h0 + 1:hi - h0 + 1, 1:W + 1] = d_var[b, 0, lo:hi, :]
        rgb_c = np.ascontiguousarray(
            rgb_b[b, :, h0:h0 + R, :].transpose(1, 0, 2)).reshape(R * C, W)
        d_c = np.ascontiguousarray(
            d_b[b, :, h0:h0 + R, :].transpose(1, 0, 2)).reshape(R * C, W)
        in_maps.append({
            "rgb_s": rgb_c, "d_s": d_c, "var_s": var.astype(NP_BF16),
            "conv_st": st, "conv_str": str_,
            "diag0": d0, "diag1": d1, "diag0r": d0r, "diag1r": d1r,
        })
    return in_maps


def run(trace=False, **inputs):
    if "nc" not in _CACHE:
        _CACHE["nc"] = _build_program()
    nc = _CACHE["nc"]
    in_maps = _shard_inputs(**inputs)
    res = run_bass_kernel_spmd(nc, in_maps, list(range(NCORES)), trace=trace)
    out = np.empty((B, C, H, W), np.float32)
    for core in range(NCORES):
        b, half = divmod(core, 2)
        o = np.asarray(res.results[core]["out_s"]).reshape(R, C, W)
        out[b, :, half * R:(half + 1) * R, :] = o.transpose(1, 0, 2).astype(np.float32)
    return out, res


def kernel(**inputs):
    out, _ = run(trace=False, **inputs)
    return out


# revision 9
# speedup vs baseline: 1.2586x; 1.0693x over previous
"""Trainium2 Bass kernel for ConditionalAttentionFusion-v2.

Math (per batch b, channel c, pixel y,x):
    CD   = concat(rgb_var, d_var)                       # [2,H,W], shared
    AB   = Wp[c,0]*rgb + Wp[c,1]*d
    CDc  = conv3x3(CD, W_unc[c])                        # 2-in 1-out per channel
    G    = Wt[c,0]*AB + Wt[c,1]*CDc
    out  = rgb*G + d*(1-G) = d + (rgb-d)*G

HBM-bound problem -> rgb/d/out move as bf16 (rel-err ~8e-3 vs the 2e-2 gate).

Layout: pure data parallel over 8 cores (core = (batch, H-half), slab of 256
rows).  Host ships rgb/d as [R, C, W] (y-major) so a "block" of 6 consecutive
image rows x all 19 channels is 114 *consecutive* rows of the flat [R*C, W]
matrix = one [114, 1024] SBUF tile with partition p = 6-local-row*19 + c.

Per 6-row block, G is accumulated entirely in PSUM by TensorE:
  - conv term: one matmul against a single reused [108,114] stationary that
    encodes all 18 (i,ky,kx) taps; the moving tile is an im2col matrix
    S2[g*18+o, x] = var[i, 6b+g+ky, x+kx], built for a whole 126-row half by
    18 big strided HBM DMAs (3.4x re-read of the tiny var tensor).
  - diag terms: bf16 diagonal-matrix matmuls diag(a0), diag(a1) against the
    rgb/d tiles themselves (a0=Wt0*Wp0, a1=Wt0*Wp1 per channel).
ScalarE drains PSUM->bf16; VectorE does 3 chunk-wide bf16 tensor_tensor ops
(2x perf mode) for out = d + (rgb-d)*G.  GPSIMD does no compute (it would
contend for the DVE SBUF port) - it only issues output-store DMAs so loads
(sync queue), d-loads/drains (scalar queue) and stores (gpsimd queue) spread
over three DMA queues.  IO DMAs are ~700KB (3 blocks per chunk).

The 4-row tail (rows 252..255) uses the same scheme with 4x19=76-partition
tiles and its own [72,76] conv stationary.
"""
import sys

if "/opt/trn_rl_repo" not in sys.path:
    sys.path.insert(0, "/opt/trn_rl_repo")

import numpy as np

import concourse.bacc as bacc
import concourse.mybir as mybir
import concourse.tile as tile
from concourse.bass_utils import run_bass_kernel_spmd

F32 = mybir.dt.float32
BF16 = mybir.dt.bfloat16
NP_BF16 = mybir.dt.np(mybir.dt.bfloat16)
ALU = mybir.AluOpType
ACT_COPY = mybir.ActivationFunctionType.Copy

B, C, H, W = 4, 19, 512, 1024
R = 256              # slab rows per core
NCORES = 8
GR = 6               # image rows per block
P = GR * C           # partitions per block = 114
NB = 42              # full blocks per slab (252 rows)
NBH = 21             # blocks per 126-row half
CB = 3               # blocks per IO chunk
REM_Y = 252          # remainder rows 252..255
PR_ = 4 * C          # remainder partitions = 76


# ----------------------------------------------------------------- host math
def _build_mats(W_prob, W_unc, W_total):
    a0 = (W_total[:, 0] * W_prob[:, 0]).astype(np.float32)
    a1 = (W_total[:, 0] * W_prob[:, 1]).astype(np.float32)
    Wp = (W_total[:, 1][:, None, None, None] * W_unc).astype(np.float32)

    # conv stationary [108, 114]: row g*18+(i*9+ky*3+kx) -> out g*19+c
    st = np.zeros((108, P), np.float32)
    for g in range(GR):
        for i in range(2):
            for ky in range(3):
                for kx in range(3):
                    o = i * 9 + ky * 3 + kx
                    st[g * 18 + o, g * 19:(g + 1) * 19] = Wp[:, i, ky, kx]
    # remainder conv stationary [72, 76]: row r*18+o -> out r*19+c
    str_ = np.zeros((72, PR_), np.float32)
    for r in range(4):
        for i in range(2):
            for ky in range(3):
                for kx in range(3):
                    o = i * 9 + ky * 3 + kx
                    str_[r * 18 + o, r * 19:(r + 1) * 19] = Wp[:, i, ky, kx]

    d0 = np.diag(np.tile(a0, GR)).astype(np.float32)         # [114,114]
    d1 = np.diag(np.tile(a1, GR)).astype(np.float32)
    d0r = np.diag(np.tile(a0, 4)).astype(np.float32)         # [76,76]
    d1r = np.diag(np.tile(a1, 4)).astype(np.float32)
    return (st.astype(NP_BF16), str_.astype(NP_BF16),
            d0.astype(NP_BF16), d1.astype(NP_BF16),
            d0r.astype(NP_BF16), d1r.astype(NP_BF16))


# ------------------------------------------------------------- bass program
_CACHE = {}


def _build_program():
    nc = bacc.Bacc("TRN2", debug=False, num_devices=NCORES)
    # chunk-major: [chunk, partition p=g*19+c, block-in-chunk, x]
    NCH = NB // CB
    rgb_s = nc.dram_tensor("rgb_s", [NCH, P, CB, W], BF16, kind="ExternalInput").ap()
    d_s = nc.dram_tensor("d_s", [NCH, P, CB, W], BF16, kind="ExternalInput").ap()
    rgb_r = nc.dram_tensor("rgb_r", [PR_, W], BF16, kind="ExternalInput").ap()
    d_r = nc.dram_tensor("d_r", [PR_, W], BF16, kind="ExternalInput").ap()
    var_s = nc.dram_tensor("var_s", [2, R + 2, W + 2], BF16, kind="ExternalInput").ap()
    st_d = nc.dram_tensor("conv_st", [108, P], BF16, kind="ExternalInput").ap()
    str_d = nc.dram_tensor("conv_str", [72, PR_], BF16, kind="ExternalInput").ap()
    d0_d = nc.dram_tensor("diag0", [P, P], BF16, kind="ExternalInput").ap()
    d1_d = nc.dram_tensor("diag1", [P, P], BF16, kind="ExternalInput").ap()
    d0r_d = nc.dram_tensor("diag0r", [PR_, PR_], BF16, kind="ExternalInput").ap()
    d1r_d = nc.dram_tensor("diag1r", [PR_, PR_], BF16, kind="ExternalInput").ap()
    out_s = nc.dram_tensor("out_s", [NCH, P, CB, W], BF16, kind="ExternalOutput").ap()
    out_r = nc.dram_tensor("out_r", [PR_, W], BF16, kind="ExternalOutput").ap()

    with tile.TileContext(nc) as tc:
        with (
            tc.tile_pool(name="wpool", bufs=1) as wpool,
            tc.tile_pool(name="spool", bufs=2) as spool,
            tc.tile_pool(name="io", bufs=3) as io,
            tc.tile_pool(name="tmp", bufs=2) as tmp,
            tc.tile_pool(name="psum", bufs=3, space="PSUM") as psum,
        ):
            st_sb = wpool.tile([108, P], BF16, tag="st", name="st_sb")
            nc.sync.dma_start(out=st_sb[:], in_=st_d[:])
            str_sb = wpool.tile([72, PR_], BF16, tag="str", name="str_sb")
            nc.sync.dma_start(out=str_sb[:], in_=str_d[:])
            d0_sb = wpool.tile([P, P], BF16, tag="d0", name="d0_sb")
            nc.sync.dma_start(out=d0_sb[:], in_=d0_d[:])
            d1_sb = wpool.tile([P, P], BF16, tag="d1", name="d1_sb")
            nc.sync.dma_start(out=d1_sb[:], in_=d1_d[:])
            d0r_sb = wpool.tile([PR_, PR_], BF16, tag="d0r", name="d0r_sb")
            nc.sync.dma_start(out=d0r_sb[:], in_=d0r_d[:])
            d1r_sb = wpool.tile([PR_, PR_], BF16, tag="d1r", name="d1r_sb")
            nc.sync.dma_start(out=d1r_sb[:], in_=d1r_d[:])

            for half in range(2):
                y0 = 126 * half
                # im2col for the half: S2[g*18+(i*9+ky*3+kx), b, x]
                #   = var_s[i, y0 + 6b + g + ky, x + kx]
                s2 = spool.tile([108, NBH, W], BF16, tag="s2", name=f"s2_{half}")
                for i in range(2):
                    for ky in range(3):
                        for kx in range(3):
                            o = i * 9 + ky * 3 + kx
                            src = var_s[i, y0 + ky:y0 + ky + 126,
                                        kx:kx + W].rearrange(
                                            "(b g) x -> g b x", g=GR)
                            eng = nc.sync if i == 0 else nc.gpsimd
                            eng.dma_start(out=s2[o:90 + o + 1:18, :, :],
                                          in_=src)
                for j in range(NBH // CB):
                    b0 = half * NBH + j * CB          # global block id
                    r0 = b0 * P                       # flat row
                    jc = b0 // CB                 # chunk id
                    rt = io.tile([P, CB, W], BF16, tag="r", name=f"r{b0}")
                    nc.sync.dma_start(out=rt[:], in_=rgb_s[jc])
                    dt = io.tile([P, CB, W], BF16, tag="d", name=f"d{b0}")
                    nc.scalar.dma_start(out=dt[:], in_=d_s[jc])

                    pss = []
                    for k in range(CB):
                        ps = psum.tile([P, W], F32, tag="ps", name=f"ps{b0 + k}")
                        pss.append(ps)
                    bl0 = j * CB                      # block index within half
                    for xb in (0, 512):
                        for k in range(CB):
                            nc.tensor.matmul(
                                pss[k][:, xb:xb + 512], st_sb[:],
                                s2[:, bl0 + k, xb:xb + 512],
                                start=True, stop=False)
                    for xb in (0, 512):
                        for k in range(CB):
                            nc.tensor.matmul(
                                pss[k][:, xb:xb + 512], d0_sb[:],
                                rt[:, k, xb:xb + 512], start=False, stop=False)
                    for xb in (0, 512):
                        for k in range(CB):
                            nc.tensor.matmul(
                                pss[k][:, xb:xb + 512], d1_sb[:],
                                dt[:, k, xb:xb + 512], start=False,
                                stop=True)
                    gt = tmp.tile([P, CB, W], BF16, tag="g", name=f"g{b0}")
                    for k in range(CB):
                        nc.scalar.activation(out=gt[:, k, :], in_=pss[k][:],
                                             func=ACT_COPY)
                    df = tmp.tile([P, CB, W], BF16, tag="df", name=f"df{b0}")
                    nc.vector.tensor_tensor(out=df[:], in0=rt[:], in1=dt[:],
                                            op=ALU.subtract)
                    pr = tmp.tile([P, CB, W], BF16, tag="pr", name=f"pr{b0}")
                    nc.vector.tensor_tensor(out=pr[:], in0=df[:], in1=gt[:],
                                            op=ALU.mult)
                    ot = io.tile([P, CB, W], BF16, tag="o", name=f"o{b0}")
                    nc.vector.tensor_tensor(out=ot[:], in0=pr[:], in1=dt[:],
                                            op=ALU.add)
                    nc.gpsimd.dma_start(out=out_s[jc], in_=ot[:])

            # ---------------- 4-row remainder (rows 252..255), p = r*19+c
            s2r = spool.tile([72, W], BF16, tag="s2r", name="s2r", bufs=1)
            for i in range(2):
                for ky in range(3):
                    for kx in range(3):
                        o = i * 9 + ky * 3 + kx
                        nc.sync.dma_start(
                            out=s2r[o:54 + o + 1:18, :],
                            in_=var_s[i, REM_Y + ky:REM_Y + ky + 4,
                                      kx:kx + W])
            rr = io.tile([PR_, W], BF16, tag="rr", name="rrem", bufs=1)
            nc.sync.dma_start(out=rr[:], in_=rgb_r[:])
            dr = io.tile([PR_, W], BF16, tag="dr", name="drem", bufs=1)
            nc.scalar.dma_start(out=dr[:], in_=d_r[:])
            ps = psum.tile([PR_, W], F32, tag="psr", name="ps_rem", bufs=1)
            for xb in (0, 512):
                nc.tensor.matmul(ps[:, xb:xb + 512], str_sb[:],
                                 s2r[:, xb:xb + 512],
                                 start=True, stop=False)
            for xb in (0, 512):
                nc.tensor.matmul(ps[:, xb:xb + 512], d0r_sb[:],
                                 rr[:, xb:xb + 512], start=False, stop=False)
            for xb in (0, 512):
                nc.tensor.matmul(ps[:, xb:xb + 512], d1r_sb[:],
                                 dr[:, xb:xb + 512], start=False,
                                 stop=True)
            gt = tmp.tile([PR_, W], BF16, tag="gr", name="g_rem", bufs=1)
            nc.scalar.activation(out=gt[:], in_=ps[:], func=ACT_COPY)
            df = tmp.tile([PR_, W], BF16, tag="dfr", name="df_rem", bufs=1)
            nc.vector.tensor_tensor(out=df[:], in0=rr[:], in1=dr[:],
                                    op=ALU.subtract)
            pr = tmp.tile([PR_, W], BF16, tag="prr", name="pr_rem", bufs=1)
            nc.vector.tensor_tensor(out=pr[:], in0=df[:], in1=gt[:],
                                    op=ALU.mult)
            ot = io.tile([PR_, W], BF16, tag="or", name="o_rem", bufs=1)
            nc.vector.tensor_tensor(out=ot[:], in0=pr[:], in1=dr[:],
                                    op=ALU.add)
            nc.gpsimd.dma_start(out=out_r[:], in_=ot[:])

    nc.compile()
    return nc


def _shard_inputs(rgb, d, rgb_var, d_var, W_prob, W_unc, W_total):
    st, str_, d0, d1, d0r, d1r = _build_mats(
        np.asarray(W_prob, np.float32),
        np.asarray(W_unc, np.float32),
        np.asarray(W_total, np.float32))
    rgb_b = np.asarray(rgb, np.float32).astype(NP_BF16)
    d_b = np.asarray(d, np.float32).astype(NP_BF16)
    in_maps = []
    for core in range(NCORES):
        b, half = divmod(core, 2)
        h0 = half * R
        var = np.zeros((2, R + 2, W + 2), np.float32)
        lo, hi = max(h0 - 1, 0), min(h0 + R + 1, H)
        var[0, lo - h0 + 1:hi - h0 + 1, 1:W + 1] = rgb_var[b, 0, lo:hi, :]
        var[1, lo - h0 + 1:hi - h0 + 1, 1:W + 1] = d_var[b, 0, lo:hi, :]
        rgb_c = rgb_b[b, :, h0:h0 + R, :].transpose(1, 0, 2).reshape(R * C, W)
        d_c = d_b[b, :, h0:h0 + R, :].transpose(1, 0, 2).reshape(R * C, W)
        # main: rows [0, NB*P) regrouped [NCH, CB, P, W] -> [NCH, P, CB, W]
        NCH = NB // CB
        rgb_m = np.ascontiguousarray(
            rgb_c[:NB * P].reshape(NCH, CB, P, W).transpose(0, 2, 1, 3))
        d_m = np.ascontiguousarray(
            d_c[:NB * P].reshape(NCH, CB, P, W).transpose(0, 2, 1, 3))
        in_maps.append({
            "rgb_s": rgb_m, "d_s": d_m,
            "rgb_r": np.ascontiguousarray(rgb_c[NB * P:]),
            "d_r": np.ascontiguousarray(d_c[NB * P:]),
            "var_s": var.astype(NP_BF16),
            "conv_st": st, "conv_str": str_,
            "diag0": d0, "diag1": d1, "diag0r": d0r, "diag1r": d1r,
        })
    return in_maps


def run(trace=False, **inputs):
    if "nc" not in _CACHE:
        _CACHE["nc"] = _build_program()
    nc = _CACHE["nc"]
    in_maps = _shard_inputs(**inputs)
    res = run_bass_kernel_spmd(nc, in_maps, list(range(NCORES)), trace=trace)
    out = np.empty((B, C, H, W), np.float32)
    for core in range(NCORES):
        b, half = divmod(core, 2)
        om = np.asarray(res.results[core]["out_s"])       # [NCH, P, CB, W]
        orr = np.asarray(res.results[core]["out_r"])      # [PR_, W]
        flat = np.empty((R * C, W), om.dtype)
        flat[:NB * P] = om.transpose(0, 2, 1, 3).reshape(NB * P, W)
        flat[NB * P:] = orr
        o = flat.reshape(R, C, W)
        out[b, :, half * R:(half + 1) * R, :] = o.transpose(1, 0, 2).astype(np.float32)
    return out, res


def kernel(**inputs):
    out, _ = run(trace=False, **inputs)
    return out


# revision 10
# speedup vs baseline: 1.3370x; 1.0624x over previous
"""Trainium2 Bass kernel for ConditionalAttentionFusion-v2.

Math (per batch b, channel c, pixel y,x):
    CD   = concat(rgb_var, d_var)                       # [2,H,W], shared
    AB   = Wp[c,0]*rgb + Wp[c,1]*d
    CDc  = conv3x3(CD, W_unc[c])                        # 2-in 1-out per channel
    G    = Wt[c,0]*AB + Wt[c,1]*CDc
    out  = rgb*G + d*(1-G) = d + (rgb-d)*G

HBM-bound problem -> rgb/d/out move as bf16 (rel-err ~8e-3 vs the 2e-2 gate).

Layout: pure data parallel over 8 cores (core = (batch, H-half), slab of 256
rows).  Host ships rgb/d as [R, C, W] (y-major) so a "block" of 6 consecutive
image rows x all 19 channels is 114 *consecutive* rows of the flat [R*C, W]
matrix = one [114, 1024] SBUF tile with partition p = 6-local-row*19 + c.

Per 6-row block, G is accumulated entirely in PSUM by TensorE:
  - conv term: one matmul against a single reused [108,114] stationary that
    encodes all 18 (i,ky,kx) taps; the moving tile is an im2col matrix
    S2[g*18+o, x] = var[i, 6b+g+ky, x+kx], built for a whole 126-row half by
    18 big strided HBM DMAs (3.4x re-read of the tiny var tensor).
  - diag terms: bf16 diagonal-matrix matmuls diag(a0), diag(a1) against the
    rgb/d tiles themselves (a0=Wt0*Wp0, a1=Wt0*Wp1 per channel).
ScalarE drains PSUM->bf16; VectorE does 3 chunk-wide bf16 tensor_tensor ops
(2x perf mode) for out = d + (rgb-d)*G.  GPSIMD does no compute (it would
contend for the DVE SBUF port) - it only issues output-store DMAs so loads
(sync queue), d-loads/drains (scalar queue) and stores (gpsimd queue) spread
over three DMA queues.  IO DMAs are ~700KB (3 blocks per chunk).

The 4-row tail (rows 252..255) uses the same scheme with 4x19=76-partition
tiles and its own [72,76] conv stationary.
"""
import sys

if "/opt/trn_rl_repo" not in sys.path:
    sys.path.insert(0, "/opt/trn_rl_repo")

import numpy as np

import concourse.bacc as bacc
import concourse.mybir as mybir
import concourse.tile as tile
from concourse.bass_utils import run_bass_kernel_spmd

F32 = mybir.dt.float32
BF16 = mybir.dt.bfloat16
NP_BF16 = mybir.dt.np(mybir.dt.bfloat16)
ALU = mybir.AluOpType
ACT_COPY = mybir.ActivationFunctionType.Copy

B, C, H, W = 4, 19, 512, 1024
R = 256              # slab rows per core
NCORES = 8
GR = 6               # image rows per block
P = GR * C           # partitions per block = 114
NB = 42              # full blocks per slab (252 rows)
NBH = 21             # blocks per 126-row half
CB = 3               # blocks per IO chunk
REM_Y = 252          # remainder rows 252..255
PR_ = 4 * C          # remainder partitions = 76


# ----------------------------------------------------------------- host math
def _build_mats(W_prob, W_unc, W_total):
    a0 = (W_total[:, 0] * W_prob[:, 0]).astype(np.float32)
    a1 = (W_total[:, 0] * W_prob[:, 1]).astype(np.float32)
    Wp = (W_total[:, 1][:, None, None, None] * W_unc).astype(np.float32)

    # conv stationary [108, 114]: row g*18+(i*9+ky*3+kx) -> out g*19+c
    st = np.zeros((108, P), np.float32)
    for g in range(GR):
        for i in range(2):
            for ky in range(3):
                for kx in range(3):
                    o = i * 9 + ky * 3 + kx
                    st[g * 18 + o, g * 19:(g + 1) * 19] = Wp[:, i, ky, kx]
    # remainder conv stationary [72, 76]: row r*18+o -> out r*19+c
    str_ = np.zeros((72, PR_), np.float32)
    for r in range(4):
        for i in range(2):
            for ky in range(3):
                for kx in range(3):
                    o = i * 9 + ky * 3 + kx
                    str_[r * 18 + o, r * 19:(r + 1) * 19] = Wp[:, i, ky, kx]

    d0 = np.diag(np.tile(a0, GR)).astype(np.float32)         # [114,114]
    d1 = np.diag(np.tile(a1, GR)).astype(np.float32)
    d0r = np.diag(np.tile(a0, 4)).astype(np.float32)         # [76,76]
    d1r = np.diag(np.tile(a1, 4)).astype(np.float32)
    return (st.astype(NP_BF16), str_.astype(NP_BF16),
            d0.astype(NP_BF16), d1.astype(NP_BF16),
            d0r.astype(NP_BF16), d1r.astype(NP_BF16))


# ------------------------------------------------------------- bass program
_CACHE = {}


def _build_program():
    nc = bacc.Bacc("TRN2", debug=False, num_devices=NCORES)
    # chunk-major: [chunk, partition p=g*19+c, block-in-chunk, x]
    NCH = NB // CB
    rgb_s = nc.dram_tensor("rgb_s", [NCH, P, CB, W], BF16, kind="ExternalInput").ap()
    d_s = nc.dram_tensor("d_s", [NCH, P, CB, W], BF16, kind="ExternalInput").ap()
    rgb_r = nc.dram_tensor("rgb_r", [PR_, W], BF16, kind="ExternalInput").ap()
    d_r = nc.dram_tensor("d_r", [PR_, W], BF16, kind="ExternalInput").ap()
    var_s = nc.dram_tensor("var_s", [2, R + 2, W + 2], BF16, kind="ExternalInput").ap()
    st_d = nc.dram_tensor("conv_st", [108, P], BF16, kind="ExternalInput").ap()
    str_d = nc.dram_tensor("conv_str", [72, PR_], BF16, kind="ExternalInput").ap()
    d0_d = nc.dram_tensor("diag0", [P, P], BF16, kind="ExternalInput").ap()
    d1_d = nc.dram_tensor("diag1", [P, P], BF16, kind="ExternalInput").ap()
    d0r_d = nc.dram_tensor("diag0r", [PR_, PR_], BF16, kind="ExternalInput").ap()
    d1r_d = nc.dram_tensor("diag1r", [PR_, PR_], BF16, kind="ExternalInput").ap()
    out_s = nc.dram_tensor("out_s", [NCH, P, CB, W], BF16, kind="ExternalOutput").ap()
    out_r = nc.dram_tensor("out_r", [PR_, W], BF16, kind="ExternalOutput").ap()

    with tile.TileContext(nc) as tc:
        with (
            tc.tile_pool(name="wpool", bufs=1) as wpool,
            tc.tile_pool(name="spool", bufs=2) as spool,
            tc.tile_pool(name="io", bufs=3) as io,
            tc.tile_pool(name="tmp", bufs=2) as tmp,
            tc.tile_pool(name="psum", bufs=3, space="PSUM") as psum,
        ):
            st_sb = wpool.tile([108, P], BF16, tag="st", name="st_sb")
            nc.sync.dma_start(out=st_sb[:], in_=st_d[:])
            str_sb = wpool.tile([72, PR_], BF16, tag="str", name="str_sb")
            nc.sync.dma_start(out=str_sb[:], in_=str_d[:])
            d0_sb = wpool.tile([P, P], BF16, tag="d0", name="d0_sb")
            nc.sync.dma_start(out=d0_sb[:], in_=d0_d[:])
            d1_sb = wpool.tile([P, P], BF16, tag="d1", name="d1_sb")
            nc.sync.dma_start(out=d1_sb[:], in_=d1_d[:])
            d0r_sb = wpool.tile([PR_, PR_], BF16, tag="d0r", name="d0r_sb")
            nc.sync.dma_start(out=d0r_sb[:], in_=d0r_d[:])
            d1r_sb = wpool.tile([PR_, PR_], BF16, tag="d1r", name="d1r_sb")
            nc.sync.dma_start(out=d1r_sb[:], in_=d1r_d[:])

            for half in range(2):
                y0 = 126 * half
                # im2col for the half: S2[g*18+(i*9+ky*3+kx), b, x]
                #   = var_s[i, y0 + 6b + g + ky, x + kx]
                s2 = spool.tile([108, NBH, W], BF16, tag="s2", name=f"s2_{half}")
                for i in range(2):
                    for ky in range(3):
                        for kx in range(3):
                            o = i * 9 + ky * 3 + kx
                            src = var_s[i, y0 + ky:y0 + ky + 126,
                                        kx:kx + W].rearrange(
                                            "(b g) x -> g b x", g=GR)
                            eng = nc.sync if i == 0 else nc.gpsimd
                            eng.dma_start(out=s2[o:90 + o + 1:18, :, :],
                                          in_=src)
                for j in range(NBH // CB):
                    b0 = half * NBH + j * CB          # global block id
                    r0 = b0 * P                       # flat row
                    jc = b0 // CB                 # chunk id
                    rt = io.tile([P, CB, W], BF16, tag="r", name=f"r{b0}", bufs=4)
                    nc.sync.dma_start(out=rt[:], in_=rgb_s[jc])
                    dt = io.tile([P, CB, W], BF16, tag="d", name=f"d{b0}", bufs=4)
                    nc.sync.dma_start(out=dt[:], in_=d_s[jc])

                    pss = []
                    for k in range(CB):
                        ps = psum.tile([P, W], F32, tag="ps", name=f"ps{b0 + k}")
                        pss.append(ps)
                    bl0 = j * CB                      # block index within half
                    for xb in (0, 512):
                        for k in range(CB):
                            nc.tensor.matmul(
                                pss[k][:, xb:xb + 512], st_sb[:],
                                s2[:, bl0 + k, xb:xb + 512],
                                start=True, stop=False)
                    for xb in (0, 512):
                        for k in range(CB):
                            nc.tensor.matmul(
                                pss[k][:, xb:xb + 512], d0_sb[:],
                                rt[:, k, xb:xb + 512], start=False, stop=False)
                    for xb in (0, 512):
                        for k in range(CB):
                            nc.tensor.matmul(
                                pss[k][:, xb:xb + 512], d1_sb[:],
                                dt[:, k, xb:xb + 512], start=False,
                                stop=True)
                    gt = tmp.tile([P, CB, W], BF16, tag="g", name=f"g{b0}", bufs=3)
                    for k in range(CB):
                        nc.scalar.activation(out=gt[:, k, :], in_=pss[k][:],
                                             func=ACT_COPY)
                    df = tmp.tile([P, CB, W], BF16, tag="df", name=f"df{b0}",
                                  bufs=4)
                    nc.vector.tensor_tensor(out=df[:], in0=rt[:], in1=dt[:],
                                            op=ALU.subtract)
                    nc.vector.tensor_tensor(out=df[:], in0=df[:], in1=gt[:],
                                            op=ALU.mult)
                    nc.vector.tensor_tensor(out=df[:], in0=df[:], in1=dt[:],
                                            op=ALU.add)
                    nc.gpsimd.dma_start(out=out_s[jc], in_=df[:])

            # ---------------- 4-row remainder (rows 252..255), p = r*19+c
            s2r = spool.tile([72, W], BF16, tag="s2r", name="s2r", bufs=1)
            for i in range(2):
                for ky in range(3):
                    for kx in range(3):
                        o = i * 9 + ky * 3 + kx
                        nc.sync.dma_start(
                            out=s2r[o:54 + o + 1:18, :],
                            in_=var_s[i, REM_Y + ky:REM_Y + ky + 4,
                                      kx:kx + W])
            rr = io.tile([PR_, W], BF16, tag="rr", name="rrem", bufs=1)
            nc.sync.dma_start(out=rr[:], in_=rgb_r[:])
            dr = io.tile([PR_, W], BF16, tag="dr", name="drem", bufs=1)
            nc.sync.dma_start(out=dr[:], in_=d_r[:])
            ps = psum.tile([PR_, W], F32, tag="psr", name="ps_rem", bufs=1)
            for xb in (0, 512):
                nc.tensor.matmul(ps[:, xb:xb + 512], str_sb[:],
                                 s2r[:, xb:xb + 512],
                                 start=True, stop=False)
            for xb in (0, 512):
                nc.tensor.matmul(ps[:, xb:xb + 512], d0r_sb[:],
                                 rr[:, xb:xb + 512], start=False, stop=False)
            for xb in (0, 512):
                nc.tensor.matmul(ps[:, xb:xb + 512], d1r_sb[:],
                                 dr[:, xb:xb + 512], start=False,
                                 stop=True)
            gt = tmp.tile([PR_, W], BF16, tag="gr", name="g_rem", bufs=1)
            nc.scalar.activation(out=gt[:], in_=ps[:], func=ACT_COPY)
            df = tmp.tile([PR_, W], BF16, tag="dfr", name="df_rem", bufs=1)
            nc.vector.tensor_tensor(out=df[:], in0=rr[:], in1=dr[:],
                                    op=ALU.subtract)
            pr = tmp.tile([PR_, W], BF16, tag="prr", name="pr_rem", bufs=1)
            nc.vector.tensor_tensor(out=pr[:], in0=df[:], in1=gt[:],
                                    op=ALU.mult)
            ot = io.tile([PR_, W], BF16, tag="or", name="o_rem", bufs=1)
            nc.vector.tensor_tensor(out=ot[:], in0=pr[:], in1=dr[:],
                                    op=ALU.add)
            nc.gpsimd.dma_start(out=out_r[:], in_=ot[:])

    nc.compile()
    return nc


def _shard_inputs(rgb, d, rgb_var, d_var, W_prob, W_unc, W_total):
    st, str_, d0, d1, d0r, d1r = _build_mats(
        np.asarray(W_prob, np.float32),
        np.asarray(W_unc, np.float32),
        np.asarray(W_total, np.float32))
    rgb_b = np.asarray(rgb, np.float32).astype(NP_BF16)
    d_b = np.asarray(d, np.float32).astype(NP_BF16)
    in_maps = []
    for core in range(NCORES):
        b, half = divmod(core, 2)
        h0 = half * R
        var = np.zeros((2, R + 2, W + 2), np.float32)
        lo, hi = max(h0 - 1, 0), min(h0 + R + 1, H)
        var[0, lo - h0 + 1:hi - h0 + 1, 1:W + 1] = rgb_var[b, 0, lo:hi, :]
        var[1, lo - h0 + 1:hi - h0 + 1, 1:W + 1] = d_var[b, 0, lo:hi, :]
        rgb_c = rgb_b[b, :, h0:h0 + R, :].transpose(1, 0, 2).reshape(R * C, W)
        d_c = d_b[b, :, h0:h0 + R, :].transpose(1, 0, 2).reshape(R * C, W)
        # main: rows [0, NB*P) regrouped [NCH, CB, P, W] -> [NCH, P, CB, W]
        NCH = NB // CB
        rgb_m = np.ascontiguousarray(
            rgb_c[:NB * P].reshape(NCH, CB, P, W).transpose(0, 2, 1, 3))
        d_m = np.ascontiguousarray(
            d_c[:NB * P].reshape(NCH, CB, P, W).transpose(0, 2, 1, 3))
        in_maps.append({
            "rgb_s": rgb_m, "d_s": d_m,
            "rgb_r": np.ascontiguousarray(rgb_c[NB * P:]),
            "d_r": np.ascontiguousarray(d_c[NB * P:]),
            "var_s": var.astype(NP_BF16),
            "conv_st": st, "conv_str": str_,
            "diag0": d0, "diag1": d1, "diag0r": d0r, "diag1r": d1r,
        })
    return in_maps


def run(trace=False, **inputs):
    if "nc" not in _CACHE:
        _CACHE["nc"] = _build_program()
    nc = _CACHE["nc"]
    in_maps = _shard_inputs(**inputs)
    res = run_bass_kernel_spmd(nc, in_maps, list(range(NCORES)), trace=trace)
    out = np.empty((B, C, H, W), np.float32)
    for core in range(NCORES):
        b, half = divmod(core, 2)
        om = np.asarray(res.results[core]["out_s"])       # [NCH, P, CB, W]
        orr = np.asarray(res.results[core]["out_r"])      # [PR_, W]
        flat = np.empty((R * C, W), om.dtype)
        flat[:NB * P] = om.transpose(0, 2, 1, 3).reshape(NB * P, W)
        flat[NB * P:] = orr
        o = flat.reshape(R, C, W)
        out[b, :, half * R:(half + 1) * R, :] = o.transpose(1, 0, 2).astype(np.float32)
    return out, res


def kernel(**inputs):
    out, _ = run(trace=False, **inputs)
    return out
